# revision 40
# baseline (speedup 1.0000x reference)
"""Causal self-attention (B=2, L=2048, E=768, H=12) on 8 trn2 NeuronCores.

Sharding: data parallel over B (cores 0-3 -> b=0, cores 4-7 -> b=1), tensor
parallel over heads (each core owns 3 heads).  Per core:
  - all inputs in bf16; weights DMA'd first, then x^T in 6 row chunks so the
    q/k projections consume chunks as they land (8 PSUM accumulators),
  - qT/kT computed directly in transposed [d, L] layout (scores = K @ Q^T),
  - scores kept TRANSPOSED S^T [keys, queries]; softmax denominators come
    from a ones-column appended to V (no max subtraction needed: |s| <~ 2),
  - scores for key-block kb+1 are emitted ahead of the numerator for kb so
    the PE never idles behind the scalar-engine exp,
  - numerator Y^T = [V|1]^T @ E^T in bf16; per-head 1/den via ones-broadcast
    matmul, multiplied straight out of PSUM,
  - output projection with h0/h1 packed into one 128-row stationary tile;
    per-chunk partials go out in bf16; ReduceScatter (bf16 add) over the 4
    cores of each batch; + bias, tanh on chip; bf16 output.
Host side only reshapes/transposes/casts inputs and concatenates the output.
"""
import hashlib
import os
import shutil

import numpy as np

import concourse.bacc as bacc
import concourse.mybir as mybir
import concourse.tile as tile
from concourse import bass_utils, bass2jax

F32 = mybir.dt.float32
F32R = mybir.dt.float32r
BF16 = mybir.dt.bfloat16
AF = mybir.ActivationFunctionType

B, L, E, H, D = 2, 2048, 768, 12, 64
# bisect flags (defaults = shipped config; env only used for tuning runs)
MASK_ON_PE = os.environ.get("K_MASK_PE", "0") == "1"
INTERLEAVE_HALVES = os.environ.get("K_INTERLEAVE", "0") == "1"
ACT_BIAS = os.environ.get("K_ACTBIAS", "1") == "1"
HPC = 3                      # heads per core
NC = 8
GROUPS = [[0, 1, 2, 3], [4, 5, 6, 7]]
EC = E // 128                # 6 embedding chunks
QC = L // 512                # 4 query chunks of 512
KB = L // 128                # 16 key blocks of 128

# ---------------------------------------------------------------------------
# NEFF compile memoization (same BIR -> same NEFF); safe, process-local.
_orig_compile = bass_utils.compile_bir_kernel
_CACHE_DIR = os.environ.get("NEFF_MEMO_DIR", "/tmp/neff_cache")


def _memo_compile(bir_json, tmpdir, neff_name="file.neff"):
    try:
        os.makedirs(_CACHE_DIR, exist_ok=True)
        key = hashlib.sha256(bir_json).hexdigest()[:24]
        cached = os.path.join(_CACHE_DIR, f"{key}.neff")
        if os.path.exists(cached):
            dst = os.path.join(tmpdir, neff_name)
            shutil.copy(cached, dst)
            return dst
        path = _orig_compile(bir_json, tmpdir, neff_name)
        shutil.copy(path, cached)
        return path
    except OSError:
        return _orig_compile(bir_json, tmpdir, neff_name)


bass_utils.compile_bir_kernel = _memo_compile
bass2jax.compile_bir_kernel = _memo_compile


# ---------------------------------------------------------------------------
def _emit_body(nc, tc, io, pools, with_collective=True):
    (xT, wqk, wv, bqk, bv, wop, wo2, bo_s, tri01, maskneg, idb, ones64, out_bt) = io
    consts, pers, work, mm, sc, num, dram = pools

    # ---- constant loads: qk weights + masks first, then x chunks, then
    # the weights that are only needed later (v proj, outproj).  All weight
    # tensors are host-prepped partition-major so each DMA is one contiguous
    # run per partition (descriptor-cheap).
    # per-slot weight tiles: the first projection matmul only needs slot 1
    wqk_t = consts.tile([128, 3, EC, 128], BF16, name="wqk_t")
    nc.sync.dma_start(out=wqk_t[:, 1], in_=wqk.ap()[:, 1])
    nc.sync.dma_start(out=wqk_t[:, 0], in_=wqk.ap()[:, 0])

    xts = []
    for c in range(EC):
        xt = consts.tile([128, L], BF16, name=f"xt{c}")
        nc.sync.dma_start(out=xt, in_=xT.ap()[128 * c:128 * c + 128, :])
        xts.append(xt)

    nc.sync.dma_start(out=wqk_t[:, 2], in_=wqk.ap()[:, 2])
    bqk_t = consts.tile([128, 3], F32, name="bqk_t")
    nc.sync.dma_start(out=bqk_t, in_=bqk.ap())
    tri_t = consts.tile([128, 128], BF16, name="tri_t")
    nc.sync.dma_start(out=tri_t, in_=tri01.ap())
    if MASK_ON_PE:
        maskn_t = consts.tile([128, 128], BF16, name="maskn_t")
        nc.sync.dma_start(out=maskn_t, in_=maskneg.ap())
        idb_t = consts.tile([128, 128], BF16, name="idb_t")
        nc.sync.dma_start(out=idb_t, in_=idb.ap())
    ones_t = consts.tile([1, 64], F32R, name="ones_t")
    nc.sync.dma_start(out=ones_t, in_=ones64.ap().bitcast(F32R))

    wv_t = consts.tile([128, EC, 256], BF16, name="wv_t")
    nc.sync.dma_start(out=wv_t, in_=wv.ap())
    bv_t = consts.tile([128, 256], F32, name="bv_t")
    nc.sync.dma_start(out=bv_t, in_=bv.ap())
    wop_t = consts.tile([128, E], BF16, name="wop_t")
    nc.sync.dma_start(out=wop_t, in_=wop.ap())
    wo2_t = consts.tile([64, E], BF16, name="wo2_t")
    nc.sync.dma_start(out=wo2_t, in_=wo2.ap())
    bo1_t = consts.tile([128, 1], F32, name="bo1_t")
    nc.sync.dma_start(out=bo1_t, in_=bo_s.ap()[0:128])
    bo2_t = consts.tile([64, 1], F32, name="bo2_t")
    nc.sync.dma_start(out=bo2_t, in_=bo_s.ap()[128:192])

    # ---- persistent tiles ----------------------------------------------
    qTp = pers.tile([128, L], BF16, name="qTp")   # h0 rows 0:64, h1 rows 64:128
    kTp = pers.tile([128, L], BF16, name="kTp")
    qkT2 = pers.tile([128, L], BF16, name="qkT2") # h2: q rows 0:64, k rows 64:128
    kT2 = pers.tile([64, L], BF16, name="kT2")    # h2 k shifted to base 0 via sb2sb DMA
    v_t = pers.tile([128, KB, 256], BF16, name="v_t")
    yTp = pers.tile([128, L], BF16, name="yTp")   # h0 rows 0:64, h1 rows 64:128
    yT2 = pers.tile([64, L], BF16, name="yT2")

    rs_ins = [dram.tile([E, 512], BF16, name=f"rs_in{j}") for j in range(QC)]
    rs_outs = [dram.tile([192, 512], BF16, name=f"rs_out{j}") for j in range(QC)]

    # ---- q/k projections for h0/h1: slots 0 (q) and 1 (k) interleaved ----
    # 8 PSUM accumulators (all 8 banks) so every chunk c is fully consumed as
    # soon as its DMA lands: both slots, all 4 j's.
    # slot 1 -> sc pool tiles (2x [128,1024]); slot 0 -> num+mm tiles.
    pk = [sc.tile([128, 1024], F32, tag="sc", name=f"qk_k_{t}") for t in range(2)]
    pq = [num.tile([128, 512], F32, tag="num", name=f"qk_q_{t}") for t in range(2)] + \
         [mm.tile([128, 512], F32, tag="mm", name=f"qk_q_{t}") for t in range(2, 4)]

    def qk_ps(slot, j):
        if slot == 1:
            return pk[j // 2][:, 512 * (j % 2):512 * (j % 2) + 512]
        return pq[j][:, 0:512]

    for c in range(EC):
        for slot in (1, 0):
            for j in range(QC):
                nc.tensor.matmul(qk_ps(slot, j), wqk_t[:, slot, c],
                                 xts[c][:, 512 * j:512 * j + 512],
                                 start=(c == 0), stop=(c == EC - 1))
    # bias-add order chosen so h0's first score blocks unblock after 3 ops;
    # the non-critical ones run on the scalar engine (idle until exps start)
    # k j0+j1 live contiguously in one PSUM tile: one wide bias-add
    nc.vector.tensor_scalar_add(out=kTp[:, 0:1024], in0=pk[0][:, 0:1024],
                                scalar1=bqk_t[:, 1:2])
    dve_set = (((0, 0), (0, 1)) if ACT_BIAS else
               ((0, 0), (0, 1), (1, 1), (1, 2), (0, 2), (1, 3), (0, 3)))
    for slot, j in dve_set:
        dst = kTp if slot == 1 else qTp
        nc.vector.tensor_scalar_add(
            out=dst[:, 512 * j:512 * j + 512],
            in0=qk_ps(slot, j),
            scalar1=bqk_t[:, slot:slot + 1])
    if ACT_BIAS:
        nc.scalar.activation(kTp[:, 1024:2048], pk[1][:, 0:1024],
                             AF.Identity, bias=bqk_t[:, 1:2])
        for slot, j in ((0, 2), (0, 3)):
            nc.scalar.activation(qTp[:, 512 * j:512 * j + 512], qk_ps(slot, j),
                                 AF.Identity, bias=bqk_t[:, slot:slot + 1])

    # ---- h2 q/k projection (slot 2); emitted under h0's attention --------
    def emit_qk2():
        p2 = [num.tile([128, 512], F32, tag="num", name=f"qk2_{t}")
              for t in range(2)] + \
             [mm.tile([128, 512], F32, tag="mm", name=f"qk2_{t}")
              for t in range(2, 4)]
        for c in range(EC):
            for j in range(QC):
                nc.tensor.matmul(p2[j][:, 0:512], wqk_t[:, 2, c],
                                 xts[c][:, 512 * j:512 * j + 512],
                                 start=(c == 0), stop=(c == EC - 1))
        for j in range(QC):
            nc.vector.tensor_scalar_add(
                out=qkT2[:, 512 * j:512 * j + 512],
                in0=p2[j][:, 0:512],
                scalar1=bqk_t[:, 2:3])
            nc.sync.dma_start(out=kT2[:, 512 * j:512 * j + 512],
                                in_=qkT2[64:128, 512 * j:512 * j + 512])

    # ---- v projection (needs all chunks; runs under h0 attention) --------
    def emit_v(lc):
        ps = mm.tile([128, 256], F32, tag="mm", name=f"ps_v{lc}")
        for c in range(EC):
            nc.tensor.matmul(ps, xts[c][:, 128 * lc:128 * lc + 128], wv_t[:, c],
                             start=(c == 0), stop=(c == EC - 1))
        nc.vector.tensor_add(v_t[:, lc, :], ps[:, :], bv_t[:, :])

    # ---- attention (per head, q-halves, kb-outer, scores 1 ahead) --------
    # yTp pairs h0+h2 (h1 separate) so the outproj's first accumulation MM
    # (the pair) is ready before h1 — the last-emitted head — normalizes.
    def yT_dst(h):
        return yTp[0:64, :] if h == 0 else (yTp[64:128, :] if h == 2 else yT2)

    def emit_head_half(h, qT, kT, half, post_chunk=None, pre_kb=None):
        h_lo, h_hi = 1024 * half, 1024 * half + 1024
        jset = (2 * half, 2 * half + 1)
        pn = {j: num.tile([65, 512], F32, tag="num", name=f"pn{h}_{j}")
              for j in jset}
        kb_end = 8 if half == 0 else 16
        pending = None      # (segs, ew) for the previous key-block unit
        norm_q = []         # chunk normalizations, delayed one extra stage

        def emit_numerator(segs, ew):
            # sorted by kb so the stop-marked matmul is last in its group
            done = set()
            for tcc, qs, w, kb in sorted(segs, key=lambda s: s[3]):
                j = qs // 512
                nc.tensor.matmul(pn[j][:, qs - 512 * j:qs - 512 * j + w],
                                 v_t[:, kb, 65 * h:65 * h + 65],
                                 ew[:, tcc:tcc + w],
                                 start=(kb == 0), stop=(kb == 4 * j + 3))
                if kb % 4 == 3 and kb // 4 in jset and kb not in done:
                    done.add(kb)
                    norm_q.append(kb // 4)

        def emit_norm(j):
            r_row = work.tile([1, 512], F32R, tag="rr", name=f"rr{h}_{j}")
            with nc.allow_low_precision(reason="f32r storage"):
                nc.vector.reciprocal(r_row, pn[j][64:65, :])
            pbc = mm.tile([64, 512], F32, tag="mm", name=f"pbc{h}_{j}")
            nc.tensor.matmul(pbc, ones_t[:], r_row, start=True, stop=True)
            b_sb = work.tile([64, 512], F32, tag="bsb", name=f"bsb{h}_{j}")
            nc.vector.tensor_copy(b_sb, pbc)
            nc.vector.tensor_mul(yT_dst(h)[:, 512 * j:512 * j + 512],
                                 pn[j][0:64, :], b_sb)
            if post_chunk is not None:
                post_chunk(j)

        # Key blocks are processed in "units" sharing one score tile and one
        # exp.  The last quad of each half has no full-width segments for
        # m=1..3, so those three ragged blocks pack gaplessly into two PSUM
        # banks: m=1 (384) @0, m=3 (128) @384, m=2 (256) @512.
        units = []
        for kb in range(kb_end):
            j0, m = kb // 4, kb % 4
            if 512 * j0 >= h_lo and 512 * (j0 + 1) == h_hi and m > 0:
                if m == 1:
                    units.append([kb])
                else:
                    units[-1].append(kb)
            else:
                units.append([kb])

        for unit in units:
            for kb in unit:
                if pre_kb is not None:
                    pre_kb(kb)
            segs = []        # (tile_col, qstart, width, kb)
            diag_tcs = []    # (tile_col, kb) of causal-triangle blocks
            if len(unit) > 1:
                j0 = unit[0] // 4
                for tcol, m in ((0, 1), (384, 3), (512, 2)):
                    segs.append((tcol, 512 * j0 + 128 * m, 512 - 128 * m,
                                 4 * j0 + m))
                    diag_tcs.append((tcol, 4 * j0 + m))
                ext = 768
            else:
                kb = unit[0]
                j0, m = kb // 4, kb % 4
                has_diag = 512 * j0 >= h_lo
                if has_diag and m > 0:
                    q0, qfull = 512 * j0 + 128 * m, 512 * (j0 + 1)
                else:
                    q0 = qfull = 512 * j0 if has_diag else h_lo
                diag_q = 512 * j0 + 128 * m
                tcol = 0
                for qs in range(qfull, h_hi, 512):
                    segs.append((tcol, qs, 512, kb))
                    if has_diag and qs == diag_q:
                        diag_tcs.append((tcol, kb))
                    tcol += 512
                if has_diag and m > 0:
                    segs.append((tcol, q0, 512 - 128 * m, kb))
                    diag_tcs.append((tcol, kb))
                    tcol += 512 - 128 * m
                ext = tcol
            uname = f"{h}_{half}_{unit[0]}"
            diag_set = set(diag_tcs)
            scw = sc.tile([128, ext], F32, tag="sc", name=f"sc{uname}")
            for tcc, qs, w, kb in segs:
                diag_here = (tcc, kb) in diag_set
                nc.tensor.matmul(scw[:, tcc:tcc + w],
                                 kT[:, 128 * kb:128 * kb + 128],
                                 qT[:, qs:qs + w],
                                 start=True,
                                 stop=not (MASK_ON_PE and diag_here))
                if MASK_ON_PE and diag_here:
                    # accumulate causal -1e30 upper-tri mask via PE
                    nc.tensor.matmul(scw[:, tcc:tcc + 128], idb_t, maskn_t,
                                     start=False, stop=True)
            ew = work.tile([128, ext], BF16, tag="et", name=f"e{uname}")
            nc.scalar.activation(ew, scw, AF.Exp)
            if not MASK_ON_PE:
                for tcc, kb in diag_tcs:
                    # zero the below-diagonal keys of the block (causal mask)
                    nc.vector.tensor_mul(ew[:, tcc:tcc + 128],
                                         ew[:, tcc:tcc + 128], tri_t)
            due, norm_q[:] = norm_q[:], []
            if pending is not None:
                emit_numerator(*pending)
            pending = (segs, ew)
            for j in due:
                emit_norm(j)
        emit_numerator(*pending)
        for j in norm_q:
            emit_norm(j)

    heads = [(qTp[0:64, :], kTp[0:64, :], 0),
             (qkT2[0:64, :], kT2, 2),
             (qTp[64:128, :], kTp[64:128, :], 1)]

    # ---- output projection + chunked ReduceScatter + bias/tanh ----------
    def emit_outproj(j):
        o_st = work.tile([128, EC, 512], BF16, tag="ost", name=f"ost_{j}")
        for me in range(EC):
            po = mm.tile([128, 512], F32, tag="mm", name=f"po{me}_{j}")
            nc.tensor.matmul(po, wop_t[:, 128 * me:128 * me + 128],
                             yTp[:, 512 * j:512 * j + 512], start=True, stop=False)
            nc.tensor.matmul(po, wo2_t[:, 128 * me:128 * me + 128],
                             yT2[:, 512 * j:512 * j + 512], start=False, stop=True)
            if j == QC - 1 and me % 2 == 0:
                # tail chunk: ACT is done with exps — split copies across engines
                nc.scalar.activation(o_st[:, me, :], po, AF.Copy)
            else:
                nc.vector.tensor_copy(o_st[:, me, :], po)
        # one DMA for the whole [768, 512] partial block
        nc.sync.dma_start(out=rs_ins[j].rearrange("(c p) m -> p c m", p=128),
                          in_=o_st)
        if with_collective:
            nc.gpsimd.collective_compute(
                "ReduceScatter", mybir.AluOpType.add, replica_groups=GROUPS,
                ins=[rs_ins[j].opt()], outs=[rs_outs[j].opt()])
            rs_o = rs_outs[j]
        else:
            rs_o = rs_ins[j][0:192, :]   # timing-only variant: skip comm
        t1 = work.tile([128, 512], BF16, tag="ot", name=f"fin1_{j}")
        nc.sync.dma_start(out=t1, in_=rs_o[0:128, :])
        f1 = work.tile([128, 512], BF16, tag="of", name=f"fo1_{j}")
        nc.scalar.activation(f1, t1, AF.Tanh, bias=bo1_t, scale=1.0)
        nc.sync.dma_start(out=out_bt.ap()[0:128, 512 * j:512 * j + 512], in_=f1)
        t2 = work.tile([64, 512], BF16, tag="ot2", name=f"fin2_{j}")
        nc.sync.dma_start(out=t2, in_=rs_o[128:192, :])
        f2 = work.tile([64, 512], BF16, tag="of2", name=f"fo2_{j}")
        nc.scalar.activation(f2, t2, AF.Tanh, bias=bo2_t, scale=1.0)
        nc.sync.dma_start(out=out_bt.ap()[128:192, 512 * j:512 * j + 512], in_=f2)

    # Half-interleaved head order: all heads finish half0 (query chunks 0/1)
    # before any head starts half1, so outproj + ReduceScatter for chunks 0/1
    # launch at the kernel's midpoint and hide under half1's compute.
    # h0 goes first (its q/k are ready earliest) and carries the v projection
    # just ahead of the numerators that consume it; h1 goes last per half and
    # triggers outproj.
    (qT0, kT0, _), (qT2, kT2v, _), (qT1, kT1, _) = heads
    if INTERLEAVE_HALVES:
        emit_head_half(0, qT0, kT0, 0, pre_kb=emit_v)
        emit_qk2()
        emit_head_half(2, qT2, kT2v, 0)
        emit_head_half(1, qT1, kT1, 0, post_chunk=emit_outproj)
        emit_head_half(0, qT0, kT0, 1,
                       pre_kb=lambda kb: emit_v(kb) if kb >= 8 else None)
        emit_head_half(2, qT2, kT2v, 1)
        emit_head_half(1, qT1, kT1, 1, post_chunk=emit_outproj)
    else:
        emit_head_half(0, qT0, kT0, 0, pre_kb=emit_v)
        emit_qk2()
        emit_head_half(0, qT0, kT0, 1,
                       pre_kb=lambda kb: emit_v(kb) if kb >= 8 else None)
        emit_head_half(2, qT2, kT2v, 0)
        emit_head_half(2, qT2, kT2v, 1)
        emit_head_half(1, qT1, kT1, 0, post_chunk=emit_outproj)
        emit_head_half(1, qT1, kT1, 1, post_chunk=emit_outproj)


def build_nc(n_iters=1, with_collective=True):
    nc = bacc.Bacc("TRN2", target_bir_lowering=False, debug=False, num_devices=NC)
    io = (
        nc.declare_dram_parameter("xT", [E, L], BF16, isOutput=False),
        nc.declare_dram_parameter("wqk", [128, 3, EC, 128], BF16, isOutput=False),
        nc.declare_dram_parameter("wv", [128, EC, 256], BF16, isOutput=False),
        nc.declare_dram_parameter("bqk", [128, 3], F32, isOutput=False),
        nc.declare_dram_parameter("bv", [128, 256], F32, isOutput=False),
        nc.declare_dram_parameter("wop", [128, E], BF16, isOutput=False),
        nc.declare_dram_parameter("wo2", [64, E], BF16, isOutput=False),
        nc.declare_dram_parameter("bo_s", [192, 1], F32, isOutput=False),
        nc.declare_dram_parameter("tri01", [128, 128], BF16, isOutput=False),
        nc.declare_dram_parameter("maskneg", [128, 128], BF16, isOutput=False),
        nc.declare_dram_parameter("idb", [128, 128], BF16, isOutput=False),
        nc.declare_dram_parameter("ones64", [1, 64], F32, isOutput=False),
        nc.declare_dram_parameter("out_bt", [192, L], BF16, isOutput=True),
    )
    with tile.TileContext(nc) as tc:
        with (
            tc.tile_pool(name="consts", bufs=1) as consts,
            tc.tile_pool(name="pers", bufs=1) as pers,
            tc.tile_pool(name="work", bufs=3) as work,
            tc.tile_pool(name="mm", bufs=2, space="PSUM") as mm,
            tc.tile_pool(name="sc", bufs=2, space="PSUM") as sc,
            tc.tile_pool(name="num", bufs=2, space="PSUM") as num,
            tc.tile_pool(name="dram", bufs=1, space="DRAM") as dram,
        ):
            pools = (consts, pers, work, mm, sc, num, dram)
            if n_iters == 1:
                _emit_body(nc, tc, io, pools, with_collective)
            else:
                with tc.For_i(0, n_iters, 1):
                    _emit_body(nc, tc, io, pools, with_collective)
    nc.finalize()
    return nc


# ---------------------------------------------------------------------------
def prep_in_maps(x, Wqkv, bqkv, Wo, bo):
    import ml_dtypes
    bf16 = ml_dtypes.bfloat16

    x = np.asarray(x, np.float32)
    Wqkv = np.asarray(Wqkv, np.float32)
    bqkv = np.asarray(bqkv, np.float32)
    Wo = np.asarray(Wo, np.float32)
    bo = np.asarray(bo, np.float32)

    tri01 = np.triu(np.ones((128, 128), np.float32)).astype(bf16)
    maskneg = np.where(np.triu(np.ones((128, 128), bool)), 0.0,
                       np.float32(-1e30)).astype(bf16)
    idb = np.eye(128, dtype=bf16)
    ones64 = np.ones((1, 64), np.float32)

    in_maps = []
    for c in range(NC):
        b, rank = divmod(c, 4)
        heads = [HPC * rank + i for i in range(HPC)]
        g0, g1, g2 = heads

        def qcol(g):
            return Wqkv[:, g * 192:g * 192 + 64] / 8.0

        def kcol(g):
            return Wqkv[:, g * 192 + 64:g * 192 + 128]

        def vcol(g):
            return Wqkv[:, g * 192 + 128:g * 192 + 192]

        wqk = np.zeros((3, E, 128), np.float32)
        wqk[0] = np.concatenate([qcol(g0), qcol(g1)], axis=1)
        wqk[1] = np.concatenate([kcol(g0), kcol(g1)], axis=1)
        wqk[2] = np.concatenate([qcol(g2), kcol(g2)], axis=1)

        wv = np.zeros((E, 256), np.float32)
        bv_row = np.zeros(256, np.float32)
        for i, g in enumerate(heads):
            wv[:, 65 * i:65 * i + 64] = vcol(g)
            bv_row[65 * i:65 * i + 64] = bqkv[g * 192 + 128:g * 192 + 192]
            bv_row[65 * i + 64] = 1.0
        bv = np.broadcast_to(bv_row, (128, 256)).copy()

        bqk = np.zeros((128, 3), np.float32)
        bqk[0:64, 0] = bqkv[g0 * 192:g0 * 192 + 64] / 8.0
        bqk[64:128, 0] = bqkv[g1 * 192:g1 * 192 + 64] / 8.0
        bqk[0:64, 1] = bqkv[g0 * 192 + 64:g0 * 192 + 128]
        bqk[64:128, 1] = bqkv[g1 * 192 + 64:g1 * 192 + 128]
        bqk[0:64, 2] = bqkv[g2 * 192:g2 * 192 + 64] / 8.0
        bqk[64:128, 2] = bqkv[g2 * 192 + 64:g2 * 192 + 128]

        wop = np.concatenate([Wo[g0 * 64:g0 * 64 + 64, :],
                              Wo[g2 * 64:g2 * 64 + 64, :]], axis=0)
        wo2 = Wo[g1 * 64:g1 * 64 + 64, :]
        bo_s = bo[192 * rank:192 * rank + 192].reshape(192, 1)

        # partition-major SBUF layouts so every weight DMA is contiguous
        wqk_pm = np.ascontiguousarray(
            wqk.reshape(3, EC, 128, 128).transpose(2, 0, 1, 3))   # [p,3,c,m]
        wv_pm = np.ascontiguousarray(
            wv.reshape(EC, 128, 256).transpose(1, 0, 2))          # [p,c,m]

        in_maps.append({
            "xT": np.ascontiguousarray(x[b].T).astype(bf16),
            "wqk": wqk_pm.astype(bf16), "wv": wv_pm.astype(bf16),
            "bqk": bqk, "bv": bv,
            "wop": np.ascontiguousarray(wop).astype(bf16),
            "wo2": np.ascontiguousarray(wo2).astype(bf16),
            "bo_s": np.ascontiguousarray(bo_s),
            "tri01": tri01, "maskneg": maskneg, "idb": idb,
            "ones64": ones64,
        })
    return in_maps


def assemble(results):
    out = np.zeros((B, L, E), np.float32)
    for b in range(B):
        cols = np.concatenate(
            [np.asarray(results[4 * b + r]["out_bt"], np.float32) for r in range(4)],
            axis=0)          # [768, L]
        out[b] = cols.T
    return out


_NC_CACHE = {}


def _get_nc(n_iters=1):
    if n_iters not in _NC_CACHE:
        _NC_CACHE[n_iters] = build_nc(n_iters)
    return _NC_CACHE[n_iters]


def kernel(x, Wqkv, bqkv, Wo, bo, train=0, **_unused):
    nc = _get_nc(1)
    in_maps = prep_in_maps(x, Wqkv, bqkv, Wo, bo)
    res = bass_utils.run_bass_kernel_spmd(nc, in_maps, core_ids=list(range(NC)))
    return assemble(res.results)
